# revision 11
# baseline (speedup 1.0000x reference)
"""Trainium2 Bass kernel for nn_Head_75118978007668.

Computes, for x:[B,S,D], concept_map(cm):[D,D,D] (B=4, S=2048, D=128):
    s[b,t] = sum_{j<t} lam^(t-j) x[b,j]          (lam = 1/1.2 decayed prefix sum)
    out[b,t,f] = sum_{d,e} x[b,t,d] * s[b,t,e] * cm[f,d,e]

Sharding: 8 cores, each owns 1024 contiguous positions of one batch row
(4 rows x 2 halves).  The scan carry across the half-split is recovered
exactly (to fp32) from a 256-position halo (lam^256 ~ 4.5e-21).

Per-core dataflow (positions tiled 8 x 128):
  - carries + s tiles: small PE matmuls.
  - pass1: Y[p, (e,f)] = xT_tile.T @ W2 (PE bf16, N=512 chunks) into
    PSUM tiles of 8 e's ([128, 8, 128] = 2 banks).
  - drain, split per 8-e group across engines:
      * mult: DVE tensor_tensor with broadcast-AP s (stride-0 over f)
        for most groups; ACT per-e scale-mults for the rest.
      * 3-level fp16 tree-add (non-aliased slices): half on DVE (2x
        fp16 mode), half on GpSimd (Pool).
      Per tile a 4-level fp16 tree merges 16 partials -> out tile; its
      emission is delayed into the next tile to hide Pool stragglers.
  where W2[d, e*128+f] = cm[f, d, e]  (host-transposed).
"""

import ml_dtypes
import numpy as np

import concourse.bass as bass
import concourse.tile as tile
from concourse import bacc, mybir
from concourse.bass import ds, ts
from concourse.bass_utils import run_bass_kernel_spmd

B, S, D = 4, 2048, 128
NCORES = 8
CHUNK = S // 2          # positions per core (1024)
NT = CHUNK // 128       # position tiles per core (8)
P = 128
HALO = 256
F32 = mybir.dt.float32
BF16 = mybir.dt.bfloat16
F32R = mybir.dt.float32r
FP16 = mybir.dt.float16

# match the reference's fp32 constant 1.2 exactly
LAM = 1.0 / np.float64(np.float32(1.2))

NG = 16                 # 8-e groups per tile
EG = 8                  # e's per group

# Per-group (mult_path, tree_engine) assignment, tunable.
#   mult: 'T' = DVE broadcast tensor_tensor, 'A' = ACT scale-mults
#   tree: 'D' = DVE, 'P' = Pool (gpsimd)
MULT = "TATTATTATTATTATT"   # 11 T, 5 A
TREE = "DPPDPDDPPDPDDPPD"   # 8 D, 8 P
assert len(MULT) == NG and len(TREE) == NG

_CACHE = {}
LAST_RESULTS = None


def _host_constants():
    k = np.arange(P, dtype=np.float64)
    i = k
    LT = np.where(i[:, None] < k[None, :], LAM ** (k[None, :] - i[:, None]), 0.0)
    powv = (LAM ** k)[None, :]                      # [1, 128]
    vw = (LAM ** (P - i))[:, None]                  # [128, 1]
    j = np.arange(HALO, dtype=np.float64)
    hw = (LAM ** (HALO - j)).reshape(2, P).T        # [128, 2]
    t = np.arange(NT, dtype=np.float64)
    M9 = np.zeros((NT, NT + 1), dtype=np.float64)
    M9[:, 0] = LAM ** (P * t)
    for tt in range(NT):
        for jj in range(tt):
            M9[tt, jj + 1] = LAM ** (P * (tt - 1 - jj))
    LT9 = M9.T                                      # [9, 8]
    f32 = np.float32
    return {
        "lt": LT.astype(f32),
        "powv": powv.astype(f32),
        "vw": vw.astype(f32),
        "hw": hw.astype(f32),
        "lt9": LT9.astype(f32),
    }


def _build_nc():
    nc = bacc.Bacc("TRN2", target_bir_lowering=False, debug=False,
                   num_devices=NCORES)
    x_d = nc.declare_dram_parameter("x", [P, NT, P], F32, isOutput=False)        # [i, t, e]
    xt_d = nc.declare_dram_parameter("xt", [P, CHUNK], BF16, isOutput=False)     # [d, p]
    halo_d = nc.declare_dram_parameter("halo", [P, 2, P], F32, isOutput=False)   # [i, u, e]
    w2_d = nc.declare_dram_parameter("w2", [P, P * P], BF16, isOutput=False)     # [d, (e,f)]
    lt_d = nc.declare_dram_parameter("lt", [P, P], F32, isOutput=False)
    pow_d = nc.declare_dram_parameter("powv", [1, P], F32, isOutput=False)
    vw_d = nc.declare_dram_parameter("vw", [P, 1], F32, isOutput=False)
    hw_d = nc.declare_dram_parameter("hw", [P, 2], F32, isOutput=False)
    lt9_d = nc.declare_dram_parameter("lt9", [NT + 1, NT], F32, isOutput=False)
    out_d = nc.declare_dram_parameter("out", [P, NT, P], F32, isOutput=True)  # [p, t, f]

    mult = mybir.AluOpType.mult

    with tile.TileContext(nc) as tc:
        with tc.tile_pool(name="consts", bufs=1) as consts:
            w2_sb = [consts.tile([P, 2048], BF16, name=f"w2_sb{i}")
                     for i in range(8)]
            xt_sb = consts.tile([P, CHUNK], BF16)
            x_sb = consts.tile([P, NT, P], F32)
            halo_sb = consts.tile([P, 2, P], F32)
            lt_sb = consts.tile([P, P], F32)
            pow_sb = consts.tile([1, P], F32)
            vw_sb = consts.tile([P, 1], F32)
            hw_sb = consts.tile([P, 2], F32)
            lt9_sb = consts.tile([NT + 1, NT], F32)
            v9_sb = consts.tile([NT + 1, P], F32)
            c0_sb = consts.tile([1, P], F32)
            va_sb = consts.tile([1, 4 * P], F32)
            vb_sb = consts.tile([1, 4 * P], F32)
            c8_sb = consts.tile([NT, P], F32)
            c_all = consts.tile([1, NT * P], F32)    # [1, (t,e)] carries
            s_sb = consts.tile([P, NT, P], F32)      # [p, t, e]
            out_sb = consts.tile([P, NT, P], F32)    # [p, t, f]

            nc.sync.dma_start(out=x_sb[:, :, :], in_=x_d[:, :, :])
            nc.sync.dma_start(out=halo_sb[:, :, :], in_=halo_d[:, :, :])
            nc.sync.dma_start(out=lt_sb[:, :], in_=lt_d[:, :])
            nc.sync.dma_start(out=pow_sb[:, :], in_=pow_d[:, :])
            nc.sync.dma_start(out=vw_sb[:, :], in_=vw_d[:, :])
            nc.sync.dma_start(out=hw_sb[:, :], in_=hw_d[:, :])
            nc.sync.dma_start(out=lt9_sb[:, :], in_=lt9_d[:, :])
            nc.sync.dma_start(out=xt_sb[:, :], in_=xt_d[:, :])
            for i in range(8):
                nc.sync.dma_start(out=w2_sb[i][:, :],
                                  in_=w2_d[:, ds(2048 * i, 2048)])

            # ---- carries: c_t = s[tile_start t] for all 8 tiles ----
            with tc.tile_pool(name="psum_c", bufs=1, space="PSUM") as psum_c:
                c0_ps = psum_c.tile([1, P], F32)
                nc.tensor.matmul(c0_ps[:, :], lhsT=hw_sb[:, 0:1],
                                 rhs=halo_sb[:, 0, :], start=True, stop=False)
                nc.tensor.matmul(c0_ps[:, :], lhsT=hw_sb[:, 1:2],
                                 rhs=halo_sb[:, 1, :], start=False, stop=True)
                vps_a = psum_c.tile([1, 4 * P], F32, tag="vps_a")
                vps_b = psum_c.tile([1, 4 * P], F32, tag="vps_b")
                nc.tensor.matmul(vps_a[:, :], lhsT=vw_sb[:, :],
                                 rhs=x_sb[:, 0:4, :], start=True, stop=True)
                nc.tensor.matmul(vps_b[:, :], lhsT=vw_sb[:, :],
                                 rhs=x_sb[:, 4:8, :], start=True, stop=True)
                nc.scalar.copy(c0_sb[:, :], c0_ps[:, :])
                nc.scalar.copy(va_sb[:, :], vps_a[:, :])
                nc.scalar.copy(vb_sb[:, :], vps_b[:, :])
                nc.sync.dma_start(out=v9_sb[0:1, :], in_=c0_sb[:, :])
                nc.sync.dma_start(out=v9_sb[1:5, :], in_=va_sb[:, :])
                nc.sync.dma_start(out=v9_sb[5:9, :], in_=vb_sb[:, :])
                c_ps = psum_c.tile([NT, P], F32, tag="c_ps")
                nc.tensor.matmul(c_ps[:, :], lhsT=lt9_sb[:, :],
                                 rhs=v9_sb[:, :], start=True, stop=True)
                nc.scalar.copy(c8_sb[:, :], c_ps[:, :])
                nc.sync.dma_start(out=c_all[:, :], in_=c8_sb[:, :])

            # ---- s tiles: s = L @ x_t + pow (x) c_t ----
            with tc.tile_pool(name="psum_s", bufs=2, space="PSUM") as psum_s:
                for t in range(NT):
                    sp = psum_s.tile([P, P], F32)
                    nc.tensor.matmul(sp[:, :], lhsT=lt_sb[:, :],
                                     rhs=x_sb[:, t, :], start=True, stop=False)
                    nc.tensor.matmul(sp[:, :], lhsT=pow_sb[:, :],
                                     rhs=c_all[:, ts(t, P)], start=False, stop=True)
                    nc.scalar.copy(s_sb[:, t, :], sp[:, :])

            # ---- main: pass1 matmuls + multi-engine drain ----
            with tc.tile_pool(name="psum_y", bufs=4, space="PSUM") as psum_y, \
                 tc.tile_pool(name="tmp_pool", bufs=10) as tmp_pool, \
                 tc.tile_pool(name="part_pool", bufs=3) as part_pool:
                for t in range(NT):
                    xt_t = xt_sb[:, ts(t, P)]
                    partials = part_pool.tile([P, NG, P], FP16)
                    for g in range(NG):
                        e0 = g * EG
                        yp = psum_y.tile([P, EG, P], F32)   # 2 banks
                        for h in range(2):
                            nc.tensor.matmul(
                                yp[:, 4 * h:4 * h + 4, :], lhsT=xt_t,
                                rhs=w2_sb[g // 2][:, ds((g % 2) * 1024 + 512 * h, 512)],
                                start=True, stop=True)
                        tmp = tmp_pool.tile([P, 15, P], FP16)
                        if MULT[g] == "T":
                            sbc = s_sb[:, t, e0:e0 + EG, None].broadcast_to(
                                (P, EG, P))
                            nc.vector.tensor_tensor(
                                out=tmp[:, 0:EG, :], in0=yp[:, :, :], in1=sbc,
                                op=mult)
                        else:
                            for j in range(EG):
                                nc.scalar.mul(tmp[:, j, :], yp[:, j, :],
                                              s_sb[:, t, e0 + j:e0 + j + 1])
                        tree = nc.vector if TREE[g] == "D" else nc.gpsimd
                        tree.tensor_add(tmp[:, 8:12, :], tmp[:, 0:4, :],
                                        tmp[:, 4:8, :])
                        tree.tensor_add(tmp[:, 12:14, :], tmp[:, 8:10, :],
                                        tmp[:, 10:12, :])
                        tree.tensor_add(partials[:, g, :], tmp[:, 12, :],
                                        tmp[:, 13, :])
                    # tile-level tree (DVE, fp16 2x; last add emits fp32)
                    pt = part_pool.tile([P, 14, P], FP16, tag="ptree")
                    nc.vector.tensor_add(pt[:, 0:8, :], partials[:, 0:8, :],
                                         partials[:, 8:16, :])
                    nc.vector.tensor_add(pt[:, 8:12, :], pt[:, 0:4, :],
                                         pt[:, 4:8, :])
                    nc.vector.tensor_add(pt[:, 12:14, :], pt[:, 8:10, :],
                                         pt[:, 10:12, :])
                    nc.vector.tensor_add(out_sb[:, t, :], pt[:, 12, :],
                                         pt[:, 13, :])
                    nc.sync.dma_start(out=out_d[:, t, :], in_=out_sb[:, t, :])

    nc.finalize()
    return nc


def _get_nc():
    if "nc" not in _CACHE:
        _CACHE["nc"] = _build_nc()
    return _CACHE["nc"]


def kernel(x, concept_map, _trace=False):
    global LAST_RESULTS
    x = np.asarray(x, dtype=np.float32)
    cm = np.asarray(concept_map, dtype=np.float32)
    assert x.shape == (B, S, D) and cm.shape == (D, D, D)

    consts = _host_constants()
    # W2[d, e*128+f] = cm[f, d, e]
    w2 = np.ascontiguousarray(
        np.transpose(cm, (1, 2, 0)).reshape(D, D * D)).astype(ml_dtypes.bfloat16)

    in_maps = []
    for core in range(NCORES):
        b, half = divmod(core, 2)
        lo = half * CHUNK
        xc = x[b, lo:lo + CHUNK]                          # [1024, 128]
        x_il = np.ascontiguousarray(
            xc.reshape(NT, P, D).transpose(1, 0, 2))
        xt = np.ascontiguousarray(xc.T).astype(ml_dtypes.bfloat16)  # [d, p]
        if half == 0:
            halo = np.zeros((P, 2, D), dtype=np.float32)
        else:
            h = x[b, lo - HALO:lo]                        # [256, 128]
            halo = np.ascontiguousarray(h.reshape(2, P, D).transpose(1, 0, 2))
        in_maps.append({
            "x": x_il, "xt": xt, "halo": halo, "w2": w2, **consts,
        })

    nc = _get_nc()
    res = run_bass_kernel_spmd(nc, in_maps, list(range(NCORES)), trace=_trace)
    LAST_RESULTS = res

    out = np.empty((B, S, D), dtype=np.float32)
    for core in range(NCORES):
        b, half = divmod(core, 2)
        o = res.results[core]["out"]                      # [p, t, f]
        out[b, half * CHUNK:(half + 1) * CHUNK] = (
            o.transpose(1, 0, 2).reshape(CHUNK, D))
    return out


# revision 12
# speedup vs baseline: 1.2006x; 1.2006x over previous
"""Trainium2 Bass kernel for nn_Head_75118978007668.

Computes, for x:[B,S,D], concept_map(cm):[D,D,D] (B=4, S=2048, D=128):
    s[b,t] = sum_{j<t} lam^(t-j) x[b,j]          (lam = 1/1.2 decayed prefix sum)
    out[b,t,f] = sum_{d,e} x[b,t,d] * s[b,t,e] * cm[f,d,e]

Sharding: 8 cores, each owns 1024 contiguous positions of one batch row
(4 rows x 2 halves).  The scan carry across the half-split is recovered
exactly (to fp32) from a 256-position halo (lam^256 ~ 4.5e-21).

Per-core dataflow (positions tiled 8 x 128):
  - carries + s tiles: small PE matmuls.
  - pass1: Y[p, (e,f)] = xT_tile.T @ W2 (PE bf16, N=512 chunks) into
    PSUM tiles of 8 e's ([128, 8, 128] = 2 banks).
  - drain, split per 8-e group across engines:
      * mult: DVE tensor_tensor with broadcast-AP s (stride-0 over f)
        for most groups; ACT per-e scale-mults for the rest.
      * 3-level fp16 tree-add (non-aliased slices): half on DVE (2x
        fp16 mode), half on GpSimd (Pool).
      Per tile a 4-level fp16 tree merges 16 partials -> out tile; its
      emission is delayed into the next tile to hide Pool stragglers.
  where W2[d, e*128+f] = cm[f, d, e]  (host-transposed).
"""

import ml_dtypes
import numpy as np

import concourse.bass as bass
import concourse.tile as tile
from concourse import bacc, mybir
from concourse.bass import ds, ts
from concourse.bass_utils import run_bass_kernel_spmd

B, S, D = 4, 2048, 128
NCORES = 8
CHUNK = S // 2          # positions per core (1024)
NT = CHUNK // 128       # position tiles per core (8)
P = 128
HALO = 256
F32 = mybir.dt.float32
BF16 = mybir.dt.bfloat16
F32R = mybir.dt.float32r
FP16 = mybir.dt.float16

# match the reference's fp32 constant 1.2 exactly
LAM = 1.0 / np.float64(np.float32(1.2))

NG = 16                 # 8-e groups per tile
EG = 8                  # e's per group

# Per-group (mult_path, tree_engine) assignment, tunable.
#   mult: 'T' = DVE broadcast tensor_tensor, 'A' = ACT scale-mults
#   tree: 'D' = DVE, 'P' = Pool (gpsimd)
MULT = "TATTATTATTATTATT"   # 11 T, 5 A
TREE = "DPPDPDDPPDPDDPPD"   # 8 D, 8 P
assert len(MULT) == NG and len(TREE) == NG

_CACHE = {}
LAST_RESULTS = None


def _host_constants():
    k = np.arange(P, dtype=np.float64)
    i = k
    LT = np.where(i[:, None] < k[None, :], LAM ** (k[None, :] - i[:, None]), 0.0)
    powv = (LAM ** k)[None, :]                      # [1, 128]
    vw = (LAM ** (P - i))[:, None]                  # [128, 1]
    j = np.arange(HALO, dtype=np.float64)
    hw = (LAM ** (HALO - j)).reshape(2, P).T        # [128, 2]
    t = np.arange(NT, dtype=np.float64)
    M9 = np.zeros((NT, NT + 1), dtype=np.float64)
    M9[:, 0] = LAM ** (P * t)
    for tt in range(NT):
        for jj in range(tt):
            M9[tt, jj + 1] = LAM ** (P * (tt - 1 - jj))
    LT9 = M9.T                                      # [9, 8]
    f32 = np.float32
    return {
        "lt": LT.astype(f32),
        "powv": powv.astype(f32),
        "vw": vw.astype(f32),
        "hw": hw.astype(f32),
        "lt9": LT9.astype(f32),
    }


def _build_nc():
    nc = bacc.Bacc("TRN2", target_bir_lowering=False, debug=False,
                   num_devices=NCORES)
    x_d = nc.declare_dram_parameter("x", [P, NT, P], F32, isOutput=False)        # [i, t, e]
    xt_d = nc.declare_dram_parameter("xt", [P, CHUNK], BF16, isOutput=False)     # [d, p]
    halo_d = nc.declare_dram_parameter("halo", [P, 2, P], F32, isOutput=False)   # [i, u, e]
    w2_d = nc.declare_dram_parameter("w2", [P, P * P], BF16, isOutput=False)     # [d, (e,f)]
    lt_d = nc.declare_dram_parameter("lt", [P, P], F32, isOutput=False)
    pow_d = nc.declare_dram_parameter("powv", [1, P], F32, isOutput=False)
    vw_d = nc.declare_dram_parameter("vw", [P, 1], F32, isOutput=False)
    hw_d = nc.declare_dram_parameter("hw", [P, 2], F32, isOutput=False)
    lt9_d = nc.declare_dram_parameter("lt9", [NT + 1, NT], F32, isOutput=False)
    out_d = nc.declare_dram_parameter("out", [P, NT, P], F32, isOutput=True)  # [p, t, f]

    mult = mybir.AluOpType.mult

    with tile.TileContext(nc) as tc:
        with tc.tile_pool(name="consts", bufs=1) as consts:
            w2_sb = [consts.tile([P, 2048], BF16, name=f"w2_sb{i}")
                     for i in range(8)]
            xt_sb = consts.tile([P, CHUNK], BF16)
            x_sb = consts.tile([P, NT, P], F32)
            halo_sb = consts.tile([P, 2, P], F32)
            lt_sb = consts.tile([P, P], F32)
            pow_sb = consts.tile([1, P], F32)
            vw_sb = consts.tile([P, 1], F32)
            hw_sb = consts.tile([P, 2], F32)
            lt9_sb = consts.tile([NT + 1, NT], F32)
            v9_sb = consts.tile([NT + 1, P], F32)
            c0_sb = consts.tile([1, P], F32)
            va_sb = consts.tile([1, 4 * P], F32)
            vb_sb = consts.tile([1, 4 * P], F32)
            c8_sb = consts.tile([NT, P], F32)
            c_all = consts.tile([1, NT * P], F32)    # [1, (t,e)] carries
            s_sb = consts.tile([P, NT, P], F32)      # [p, t, e]
            out_sb = consts.tile([P, NT, P], F32)    # [p, t, f]

            nc.sync.dma_start(out=x_sb[:, :, :], in_=x_d[:, :, :])
            nc.sync.dma_start(out=halo_sb[:, :, :], in_=halo_d[:, :, :])
            nc.sync.dma_start(out=lt_sb[:, :], in_=lt_d[:, :])
            nc.sync.dma_start(out=pow_sb[:, :], in_=pow_d[:, :])
            nc.sync.dma_start(out=vw_sb[:, :], in_=vw_d[:, :])
            nc.sync.dma_start(out=hw_sb[:, :], in_=hw_d[:, :])
            nc.sync.dma_start(out=lt9_sb[:, :], in_=lt9_d[:, :])
            nc.sync.dma_start(out=xt_sb[:, :], in_=xt_d[:, :])
            for i in range(8):
                nc.sync.dma_start(out=w2_sb[i][:, :],
                                  in_=w2_d[:, ds(2048 * i, 2048)])

            # ---- carries: c_t = s[tile_start t] for all 8 tiles ----
            with tc.tile_pool(name="psum_c", bufs=1, space="PSUM") as psum_c:
                c0_ps = psum_c.tile([1, P], F32)
                nc.tensor.matmul(c0_ps[:, :], lhsT=hw_sb[:, 0:1],
                                 rhs=halo_sb[:, 0, :], start=True, stop=False)
                nc.tensor.matmul(c0_ps[:, :], lhsT=hw_sb[:, 1:2],
                                 rhs=halo_sb[:, 1, :], start=False, stop=True)
                vps_a = psum_c.tile([1, 4 * P], F32, tag="vps_a")
                vps_b = psum_c.tile([1, 4 * P], F32, tag="vps_b")
                nc.tensor.matmul(vps_a[:, :], lhsT=vw_sb[:, :],
                                 rhs=x_sb[:, 0:4, :], start=True, stop=True)
                nc.tensor.matmul(vps_b[:, :], lhsT=vw_sb[:, :],
                                 rhs=x_sb[:, 4:8, :], start=True, stop=True)
                nc.scalar.copy(c0_sb[:, :], c0_ps[:, :])
                nc.scalar.copy(va_sb[:, :], vps_a[:, :])
                nc.scalar.copy(vb_sb[:, :], vps_b[:, :])
                nc.sync.dma_start(out=v9_sb[0:1, :], in_=c0_sb[:, :])
                nc.sync.dma_start(out=v9_sb[1:5, :], in_=va_sb[:, :])
                nc.sync.dma_start(out=v9_sb[5:9, :], in_=vb_sb[:, :])
                c_ps = psum_c.tile([NT, P], F32, tag="c_ps")
                nc.tensor.matmul(c_ps[:, :], lhsT=lt9_sb[:, :],
                                 rhs=v9_sb[:, :], start=True, stop=True)
                nc.scalar.copy(c8_sb[:, :], c_ps[:, :])
                nc.sync.dma_start(out=c_all[:, :], in_=c8_sb[:, :])

            # ---- s tiles: s = L @ x_t + pow (x) c_t ----
            with tc.tile_pool(name="psum_s", bufs=2, space="PSUM") as psum_s:
                for t in range(NT):
                    sp = psum_s.tile([P, P], F32)
                    nc.tensor.matmul(sp[:, :], lhsT=lt_sb[:, :],
                                     rhs=x_sb[:, t, :], start=True, stop=False)
                    nc.tensor.matmul(sp[:, :], lhsT=pow_sb[:, :],
                                     rhs=c_all[:, ts(t, P)], start=False, stop=True)
                    nc.scalar.copy(s_sb[:, t, :], sp[:, :])

            # ---- main: pass1 matmuls + multi-engine drain ----
            with tc.tile_pool(name="psum_y", bufs=4, space="PSUM") as psum_y, \
                 tc.tile_pool(name="tmp_pool", bufs=6) as tmp_pool, \
                 tc.tile_pool(name="part_pool", bufs=2) as part_pool:
                for t in range(NT):
                    xt_t = xt_sb[:, ts(t, P)]
                    partials = part_pool.tile([P, NG, P], FP16)
                    for g in range(NG):
                        e0 = g * EG
                        yp = psum_y.tile([P, EG, P], F32)   # 2 banks
                        for h in range(2):
                            nc.tensor.matmul(
                                yp[:, 4 * h:4 * h + 4, :], lhsT=xt_t,
                                rhs=w2_sb[g // 2][:, ds((g % 2) * 1024 + 512 * h, 512)],
                                start=True, stop=True)
                        tmp = tmp_pool.tile([P, 15, P], FP16)
                        if MULT[g] == "T":
                            sbc = s_sb[:, t, e0:e0 + EG, None].broadcast_to(
                                (P, EG, P))
                            nc.vector.tensor_tensor(
                                out=tmp[:, 0:EG, :], in0=yp[:, :, :], in1=sbc,
                                op=mult)
                        else:
                            for j in range(EG):
                                nc.scalar.mul(tmp[:, j, :], yp[:, j, :],
                                              s_sb[:, t, e0 + j:e0 + j + 1])
                        tree = nc.vector if TREE[g] == "D" else nc.gpsimd
                        tree.tensor_add(tmp[:, 8:12, :], tmp[:, 0:4, :],
                                        tmp[:, 4:8, :])
                        tree.tensor_add(tmp[:, 12:14, :], tmp[:, 8:10, :],
                                        tmp[:, 10:12, :])
                        tree.tensor_add(partials[:, g, :], tmp[:, 12, :],
                                        tmp[:, 13, :])
                    # tile-level tree (DVE, fp16 2x; last add emits fp32)
                    pt = part_pool.tile([P, 14, P], FP16, tag="ptree")
                    nc.vector.tensor_add(pt[:, 0:8, :], partials[:, 0:8, :],
                                         partials[:, 8:16, :])
                    nc.vector.tensor_add(pt[:, 8:12, :], pt[:, 0:4, :],
                                         pt[:, 4:8, :])
                    nc.vector.tensor_add(pt[:, 12:14, :], pt[:, 8:10, :],
                                         pt[:, 10:12, :])
                    nc.vector.tensor_add(out_sb[:, t, :], pt[:, 12, :],
                                         pt[:, 13, :])
                    nc.sync.dma_start(out=out_d[:, t, :], in_=out_sb[:, t, :])

    nc.finalize()
    return nc


def _get_nc():
    if "nc" not in _CACHE:
        _CACHE["nc"] = _build_nc()
    return _CACHE["nc"]


def kernel(x, concept_map, _trace=False):
    global LAST_RESULTS
    x = np.asarray(x, dtype=np.float32)
    cm = np.asarray(concept_map, dtype=np.float32)
    assert x.shape == (B, S, D) and cm.shape == (D, D, D)

    consts = _host_constants()
    # W2[d, e*128+f] = cm[f, d, e]
    w2 = np.ascontiguousarray(
        np.transpose(cm, (1, 2, 0)).reshape(D, D * D)).astype(ml_dtypes.bfloat16)

    in_maps = []
    for core in range(NCORES):
        b, half = divmod(core, 2)
        lo = half * CHUNK
        xc = x[b, lo:lo + CHUNK]                          # [1024, 128]
        x_il = np.ascontiguousarray(
            xc.reshape(NT, P, D).transpose(1, 0, 2))
        xt = np.ascontiguousarray(xc.T).astype(ml_dtypes.bfloat16)  # [d, p]
        if half == 0:
            halo = np.zeros((P, 2, D), dtype=np.float32)
        else:
            h = x[b, lo - HALO:lo]                        # [256, 128]
            halo = np.ascontiguousarray(h.reshape(2, P, D).transpose(1, 0, 2))
        in_maps.append({
            "x": x_il, "xt": xt, "halo": halo, "w2": w2, **consts,
        })

    nc = _get_nc()
    res = run_bass_kernel_spmd(nc, in_maps, list(range(NCORES)), trace=_trace)
    LAST_RESULTS = res

    out = np.empty((B, S, D), dtype=np.float32)
    for core in range(NCORES):
        b, half = divmod(core, 2)
        o = res.results[core]["out"]                      # [p, t, f]
        out[b, half * CHUNK:(half + 1) * CHUNK] = (
            o.transpose(1, 0, 2).reshape(CHUNK, D))
    return out


# revision 14
# speedup vs baseline: 1.2109x; 1.0086x over previous
"""Trainium2 Bass kernel for nn_Head_75118978007668.

Computes, for x:[B,S,D], concept_map(cm):[D,D,D] (B=4, S=2048, D=128):
    s[b,t] = sum_{j<t} lam^(t-j) x[b,j]          (lam = 1/1.2 decayed prefix sum)
    out[b,t,f] = sum_{d,e} x[b,t,d] * s[b,t,e] * cm[f,d,e]

Sharding: 8 cores, each owns 1024 contiguous positions of one batch row
(4 rows x 2 halves).  The scan carry across the half-split is recovered
exactly (to fp32) from a 256-position halo (lam^256 ~ 4.5e-21).

Per-core dataflow (positions tiled 8 x 128):
  - carries + s tiles: small PE matmuls.
  - pass1: Y[p, (e,f)] = xT_tile.T @ W2 (PE bf16, N=512 chunks) into
    PSUM tiles of 8 e's ([128, 8, 128] = 2 banks).
  - drain, split per 8-e group across engines:
      * mult: DVE tensor_tensor with broadcast-AP s (stride-0 over f)
        for most groups; ACT per-e scale-mults for the rest.
      * 3-level fp16 tree-add (non-aliased slices): half on DVE (2x
        fp16 mode), half on GpSimd (Pool).
      Per tile a 4-level fp16 tree merges 16 partials -> out tile; its
      emission is delayed into the next tile to hide Pool stragglers.
  where W2[d, e*128+f] = cm[f, d, e]  (host-transposed).
"""

import ml_dtypes
import numpy as np

import concourse.bass as bass
import concourse.tile as tile
from concourse import bacc, mybir
from concourse.bass import ds, ts
from concourse.bass_utils import run_bass_kernel_spmd

B, S, D = 4, 2048, 128
NCORES = 8
CHUNK = S // 2          # positions per core (1024)
NT = CHUNK // 128       # position tiles per core (8)
P = 128
HALO = 256
F32 = mybir.dt.float32
BF16 = mybir.dt.bfloat16
F32R = mybir.dt.float32r
FP16 = mybir.dt.float16

# match the reference's fp32 constant 1.2 exactly
LAM = 1.0 / np.float64(np.float32(1.2))

NG = 16                 # 8-e groups per tile
EG = 8                  # e's per group

# Per-group (mult_path, tree_engine) assignment, tunable.
#   mult: 'T' = DVE broadcast tensor_tensor, 'A' = ACT scale-mults
#   tree: 'D' = DVE, 'P' = Pool (gpsimd)
MULT = "TATTATTATTATTATT"   # 11 T, 5 A
TREE = "DPPDPDDPPDPDDPPD"   # 8 D, 8 P
assert len(MULT) == NG and len(TREE) == NG

_CACHE = {}
LAST_RESULTS = None


def _host_constants():
    k = np.arange(P, dtype=np.float64)
    i = k
    LT = np.where(i[:, None] < k[None, :], LAM ** (k[None, :] - i[:, None]), 0.0)
    powv = (LAM ** k)[None, :]                      # [1, 128]
    vw = (LAM ** (P - i))[:, None]                  # [128, 1]
    j = np.arange(HALO, dtype=np.float64)
    hw = (LAM ** (HALO - j)).reshape(2, P).T        # [128, 2]
    t = np.arange(NT, dtype=np.float64)
    M9 = np.zeros((NT, NT + 1), dtype=np.float64)
    M9[:, 0] = LAM ** (P * t)
    for tt in range(NT):
        for jj in range(tt):
            M9[tt, jj + 1] = LAM ** (P * (tt - 1 - jj))
    LT9 = M9.T                                      # [9, 8]
    f32 = np.float32
    return {
        "lt": LT.astype(f32),
        "powv": powv.astype(f32),
        "vw": vw.astype(f32),
        "hw": hw.astype(f32),
        "lt9": LT9.astype(f32),
    }


def _build_nc():
    nc = bacc.Bacc("TRN2", target_bir_lowering=False, debug=False,
                   num_devices=NCORES)
    x_d = nc.declare_dram_parameter("x", [P, NT, P], F32, isOutput=False)        # [i, t, e]
    xt_d = nc.declare_dram_parameter("xt", [P, CHUNK], BF16, isOutput=False)     # [d, p]
    halo_d = nc.declare_dram_parameter("halo", [P, 2, P], F32, isOutput=False)   # [i, u, e]
    w2_d = nc.declare_dram_parameter("w2", [P, P * P], BF16, isOutput=False)     # [d, (e,f)]
    lt_d = nc.declare_dram_parameter("lt", [P, P], F32, isOutput=False)
    pow_d = nc.declare_dram_parameter("powv", [1, P], F32, isOutput=False)
    vw_d = nc.declare_dram_parameter("vw", [P, 1], F32, isOutput=False)
    hw_d = nc.declare_dram_parameter("hw", [P, 2], F32, isOutput=False)
    lt9_d = nc.declare_dram_parameter("lt9", [NT + 1, NT], F32, isOutput=False)
    out_d = nc.declare_dram_parameter("out", [P, NT, P], F32, isOutput=True)  # [p, t, f]

    mult = mybir.AluOpType.mult

    with tile.TileContext(nc) as tc:
        with tc.tile_pool(name="consts", bufs=1) as consts:
            w2_sb = [consts.tile([P, 2048], BF16, name=f"w2_sb{i}")
                     for i in range(8)]
            xt_sb = consts.tile([P, CHUNK], BF16)
            x_sb = consts.tile([P, NT, P], F32)
            halo_sb = consts.tile([P, 2, P], F32)
            lt_sb = consts.tile([P, P], F32)
            pow_sb = consts.tile([1, P], F32)
            vw_sb = consts.tile([P, 1], F32)
            hw_sb = consts.tile([P, 2], F32)
            lt9_sb = consts.tile([NT + 1, NT], F32)
            v9_sb = consts.tile([NT + 1, P], F32)
            c0_sb = consts.tile([1, P], F32)
            va_sb = consts.tile([1, 4 * P], F32)
            vb_sb = consts.tile([1, 4 * P], F32)
            c8_sb = consts.tile([NT, P], F32)
            c_all = consts.tile([1, NT * P], F32)    # [1, (t,e)] carries
            s_sb = consts.tile([P, NT, P], F32)      # [p, t, e]
            out_sb = consts.tile([P, NT, P], F32)    # [p, t, f]

            nc.sync.dma_start(out=x_sb[:, :, :], in_=x_d[:, :, :])
            nc.sync.dma_start(out=halo_sb[:, :, :], in_=halo_d[:, :, :])
            nc.sync.dma_start(out=lt_sb[:, :], in_=lt_d[:, :])
            nc.sync.dma_start(out=pow_sb[:, :], in_=pow_d[:, :])
            nc.sync.dma_start(out=vw_sb[:, :], in_=vw_d[:, :])
            nc.sync.dma_start(out=hw_sb[:, :], in_=hw_d[:, :])
            nc.sync.dma_start(out=lt9_sb[:, :], in_=lt9_d[:, :])
            nc.sync.dma_start(out=xt_sb[:, :], in_=xt_d[:, :])
            for i in range(8):
                nc.sync.dma_start(out=w2_sb[i][:, :],
                                  in_=w2_d[:, ds(2048 * i, 2048)])

            # ---- carries: c_t = s[tile_start t] for all 8 tiles ----
            with tc.tile_pool(name="psum_c", bufs=1, space="PSUM") as psum_c:
                c0_ps = psum_c.tile([1, P], F32)
                nc.tensor.matmul(c0_ps[:, :], lhsT=hw_sb[:, 0:1],
                                 rhs=halo_sb[:, 0, :], start=True, stop=False)
                nc.tensor.matmul(c0_ps[:, :], lhsT=hw_sb[:, 1:2],
                                 rhs=halo_sb[:, 1, :], start=False, stop=True)
                vps_a = psum_c.tile([1, 4 * P], F32, tag="vps_a")
                vps_b = psum_c.tile([1, 4 * P], F32, tag="vps_b")
                nc.tensor.matmul(vps_a[:, :], lhsT=vw_sb[:, :],
                                 rhs=x_sb[:, 0:4, :], start=True, stop=True)
                nc.tensor.matmul(vps_b[:, :], lhsT=vw_sb[:, :],
                                 rhs=x_sb[:, 4:8, :], start=True, stop=True)
                nc.scalar.copy(c0_sb[:, :], c0_ps[:, :])
                nc.scalar.copy(va_sb[:, :], vps_a[:, :])
                nc.scalar.copy(vb_sb[:, :], vps_b[:, :])
                nc.scalar.dma_start(out=v9_sb[0:1, :], in_=c0_sb[:, :])
                nc.scalar.dma_start(out=v9_sb[1:5, :], in_=va_sb[:, :])
                nc.scalar.dma_start(out=v9_sb[5:9, :], in_=vb_sb[:, :])
                c_ps = psum_c.tile([NT, P], F32, tag="c_ps")
                nc.tensor.matmul(c_ps[:, :], lhsT=lt9_sb[:, :],
                                 rhs=v9_sb[:, :], start=True, stop=True)
                nc.scalar.copy(c8_sb[:, :], c_ps[:, :])
                nc.scalar.dma_start(out=c_all[:, :], in_=c8_sb[:, :])

            # ---- s tiles: s = L @ x_t + pow (x) c_t ----
            with tc.tile_pool(name="psum_s", bufs=2, space="PSUM") as psum_s:
                for t in range(NT):
                    sp = psum_s.tile([P, P], F32)
                    nc.tensor.matmul(sp[:, :], lhsT=lt_sb[:, :],
                                     rhs=x_sb[:, t, :], start=True, stop=False)
                    nc.tensor.matmul(sp[:, :], lhsT=pow_sb[:, :],
                                     rhs=c_all[:, ts(t, P)], start=False, stop=True)
                    nc.scalar.copy(s_sb[:, t, :], sp[:, :])

            # ---- main: pass1 matmuls + multi-engine drain ----
            with tc.tile_pool(name="psum_y", bufs=4, space="PSUM") as psum_y, \
                 tc.tile_pool(name="tmp_pool", bufs=6) as tmp_pool, \
                 tc.tile_pool(name="part_pool", bufs=2) as part_pool:
                for t in range(NT):
                    xt_t = xt_sb[:, ts(t, P)]
                    partials = part_pool.tile([P, NG, P], FP16)
                    for g in range(NG):
                        e0 = g * EG
                        yp = psum_y.tile([P, EG, P], F32)   # 2 banks
                        for h in range(2):
                            nc.tensor.matmul(
                                yp[:, 4 * h:4 * h + 4, :], lhsT=xt_t,
                                rhs=w2_sb[g // 2][:, ds((g % 2) * 1024 + 512 * h, 512)],
                                start=True, stop=True)
                        tmp = tmp_pool.tile([P, 15, P], FP16)
                        if MULT[g] == "T":
                            sbc = s_sb[:, t, e0:e0 + EG, None].broadcast_to(
                                (P, EG, P))
                            nc.vector.tensor_tensor(
                                out=tmp[:, 0:EG, :], in0=yp[:, :, :], in1=sbc,
                                op=mult)
                        else:
                            for j in range(EG):
                                nc.scalar.mul(tmp[:, j, :], yp[:, j, :],
                                              s_sb[:, t, e0 + j:e0 + j + 1])
                        tree = nc.vector if TREE[g] == "D" else nc.gpsimd
                        tree.tensor_add(tmp[:, 8:12, :], tmp[:, 0:4, :],
                                        tmp[:, 4:8, :])
                        tree.tensor_add(tmp[:, 12:14, :], tmp[:, 8:10, :],
                                        tmp[:, 10:12, :])
                        tree.tensor_add(partials[:, g, :], tmp[:, 12, :],
                                        tmp[:, 13, :])
                    # tile-level tree (DVE, fp16 2x; last add emits fp32)
                    pt = part_pool.tile([P, 14, P], FP16, tag="ptree")
                    nc.vector.tensor_add(pt[:, 0:8, :], partials[:, 0:8, :],
                                         partials[:, 8:16, :])
                    nc.vector.tensor_add(pt[:, 8:12, :], pt[:, 0:4, :],
                                         pt[:, 4:8, :])
                    nc.vector.tensor_add(pt[:, 12:14, :], pt[:, 8:10, :],
                                         pt[:, 10:12, :])
                    nc.vector.tensor_add(out_sb[:, t, :], pt[:, 12, :],
                                         pt[:, 13, :])
                    nc.sync.dma_start(out=out_d[:, t, :], in_=out_sb[:, t, :])

    nc.finalize()
    return nc


def _get_nc():
    if "nc" not in _CACHE:
        _CACHE["nc"] = _build_nc()
    return _CACHE["nc"]


def kernel(x, concept_map, _trace=False):
    global LAST_RESULTS
    x = np.asarray(x, dtype=np.float32)
    cm = np.asarray(concept_map, dtype=np.float32)
    assert x.shape == (B, S, D) and cm.shape == (D, D, D)

    consts = _host_constants()
    # W2[d, e*128+f] = cm[f, d, e]
    w2 = np.ascontiguousarray(
        np.transpose(cm, (1, 2, 0)).reshape(D, D * D)).astype(ml_dtypes.bfloat16)

    in_maps = []
    for core in range(NCORES):
        b, half = divmod(core, 2)
        lo = half * CHUNK
        xc = x[b, lo:lo + CHUNK]                          # [1024, 128]
        x_il = np.ascontiguousarray(
            xc.reshape(NT, P, D).transpose(1, 0, 2))
        xt = np.ascontiguousarray(xc.T).astype(ml_dtypes.bfloat16)  # [d, p]
        if half == 0:
            halo = np.zeros((P, 2, D), dtype=np.float32)
        else:
            h = x[b, lo - HALO:lo]                        # [256, 128]
            halo = np.ascontiguousarray(h.reshape(2, P, D).transpose(1, 0, 2))
        in_maps.append({
            "x": x_il, "xt": xt, "halo": halo, "w2": w2, **consts,
        })

    nc = _get_nc()
    res = run_bass_kernel_spmd(nc, in_maps, list(range(NCORES)), trace=_trace)
    LAST_RESULTS = res

    out = np.empty((B, S, D), dtype=np.float32)
    for core in range(NCORES):
        b, half = divmod(core, 2)
        o = res.results[core]["out"]                      # [p, t, f]
        out[b, half * CHUNK:(half + 1) * CHUNK] = (
            o.transpose(1, 0, 2).reshape(CHUNK, D))
    return out


# revision 15
# speedup vs baseline: 1.2558x; 1.0370x over previous
"""Trainium2 Bass kernel for nn_Head_75118978007668.

Computes, for x:[B,S,D], concept_map(cm):[D,D,D] (B=4, S=2048, D=128):
    s[b,t] = sum_{j<t} lam^(t-j) x[b,j]          (lam = 1/1.2 decayed prefix sum)
    out[b,t,f] = sum_{d,e} x[b,t,d] * s[b,t,e] * cm[f,d,e]

Sharding: 8 cores, each owns 1024 contiguous positions of one batch row
(4 rows x 2 halves).  The scan carry across the half-split is recovered
exactly (to fp32) from a 256-position halo (lam^256 ~ 4.5e-21).

Per-core dataflow (positions tiled 8 x 128):
  - carries + s tiles: small PE matmuls.
  - pass1: Y[p, (e,f)] = xT_tile.T @ W2 (PE bf16, N=512 chunks) into
    PSUM tiles of 8 e's ([128, 8, 128] = 2 banks).
  - drain, split per 8-e group across engines:
      * mult: DVE tensor_tensor with broadcast-AP s (stride-0 over f)
        for most groups; ACT per-e scale-mults for the rest.
      * 3-level fp16 tree-add (non-aliased slices): half on DVE (2x
        fp16 mode), half on GpSimd (Pool).
      Per tile a 4-level fp16 tree merges 16 partials -> out tile; its
      emission is delayed into the next tile to hide Pool stragglers.
  where W2[d, e*128+f] = cm[f, d, e]  (host-transposed).
"""

import ml_dtypes
import numpy as np

import concourse.bass as bass
import concourse.tile as tile
from concourse import bacc, mybir
from concourse.bass import ds, ts
from concourse.bass_utils import run_bass_kernel_spmd

B, S, D = 4, 2048, 128
NCORES = 8
CHUNK = S // 2          # positions per core (1024)
NT = CHUNK // 128       # position tiles per core (8)
P = 128
HALO = 256
F32 = mybir.dt.float32
BF16 = mybir.dt.bfloat16
F32R = mybir.dt.float32r
FP16 = mybir.dt.float16

# match the reference's fp32 constant 1.2 exactly
LAM = 1.0 / np.float64(np.float32(1.2))

NG = 16                 # 8-e groups per tile
EG = 8                  # e's per group

# Per-group (mult_path, tree_engine) assignment, tunable.
#   mult: 'T' = DVE broadcast tensor_tensor, 'A' = ACT scale-mults
#   tree: 'D' = DVE, 'P' = Pool (gpsimd)
MULT = "TATTATTATTATTATT"   # 11 T, 5 A
TREE = "DPPDPPDPPDPDDPPD"   # 7 D, 9 P
assert len(MULT) == NG and len(TREE) == NG

_CACHE = {}
LAST_RESULTS = None


def _host_constants():
    k = np.arange(P, dtype=np.float64)
    i = k
    LT = np.where(i[:, None] < k[None, :], LAM ** (k[None, :] - i[:, None]), 0.0)
    powv = (LAM ** k)[None, :]                      # [1, 128]
    vw = (LAM ** (P - i))[:, None]                  # [128, 1]
    j = np.arange(HALO, dtype=np.float64)
    hw = (LAM ** (HALO - j)).reshape(2, P).T        # [128, 2]
    t = np.arange(NT, dtype=np.float64)
    M9 = np.zeros((NT, NT + 1), dtype=np.float64)
    M9[:, 0] = LAM ** (P * t)
    for tt in range(NT):
        for jj in range(tt):
            M9[tt, jj + 1] = LAM ** (P * (tt - 1 - jj))
    LT9 = M9.T                                      # [9, 8]
    f32 = np.float32
    return {
        "lt": LT.astype(f32),
        "powv": powv.astype(f32),
        "vw": vw.astype(f32),
        "hw": hw.astype(f32),
        "lt9": LT9.astype(f32),
    }


def _build_nc():
    nc = bacc.Bacc("TRN2", target_bir_lowering=False, debug=False,
                   num_devices=NCORES)
    x_d = nc.declare_dram_parameter("x", [P, NT, P], F32, isOutput=False)        # [i, t, e]
    xt_d = nc.declare_dram_parameter("xt", [P, CHUNK], BF16, isOutput=False)     # [d, p]
    halo_d = nc.declare_dram_parameter("halo", [P, 2, P], F32, isOutput=False)   # [i, u, e]
    w2_d = nc.declare_dram_parameter("w2", [P, P * P], BF16, isOutput=False)     # [d, (e,f)]
    lt_d = nc.declare_dram_parameter("lt", [P, P], F32, isOutput=False)
    pow_d = nc.declare_dram_parameter("powv", [1, P], F32, isOutput=False)
    vw_d = nc.declare_dram_parameter("vw", [P, 1], F32, isOutput=False)
    hw_d = nc.declare_dram_parameter("hw", [P, 2], F32, isOutput=False)
    lt9_d = nc.declare_dram_parameter("lt9", [NT + 1, NT], F32, isOutput=False)
    out_d = nc.declare_dram_parameter("out", [P, NT, P], F32, isOutput=True)  # [p, t, f]

    mult = mybir.AluOpType.mult

    with tile.TileContext(nc) as tc:
        with tc.tile_pool(name="consts", bufs=1) as consts:
            w2_sb = [consts.tile([P, 2048], BF16, name=f"w2_sb{i}")
                     for i in range(8)]
            xt_sb = consts.tile([P, CHUNK], BF16)
            x_sb = consts.tile([P, NT, P], F32)
            halo_sb = consts.tile([P, 2, P], F32)
            lt_sb = consts.tile([P, P], F32)
            pow_sb = consts.tile([1, P], F32)
            vw_sb = consts.tile([P, 1], F32)
            hw_sb = consts.tile([P, 2], F32)
            lt9_sb = consts.tile([NT + 1, NT], F32)
            v9_sb = consts.tile([NT + 1, P], F32)
            c0_sb = consts.tile([1, P], F32)
            va_sb = consts.tile([1, 4 * P], F32)
            vb_sb = consts.tile([1, 4 * P], F32)
            c8_sb = consts.tile([NT, P], F32)
            c_all = consts.tile([1, NT * P], F32)    # [1, (t,e)] carries
            s_sb = consts.tile([P, NT, P], F32)      # [p, t, e]
            out_sb = consts.tile([P, NT, P], F32)    # [p, t, f]

            nc.sync.dma_start(out=halo_sb[:, :, :], in_=halo_d[:, :, :])
            nc.sync.dma_start(out=hw_sb[:, :], in_=hw_d[:, :])
            nc.sync.dma_start(out=vw_sb[:, :], in_=vw_d[:, :])
            nc.sync.dma_start(out=lt9_sb[:, :], in_=lt9_d[:, :])
            nc.sync.dma_start(out=x_sb[:, :, :], in_=x_d[:, :, :])
            nc.sync.dma_start(out=lt_sb[:, :], in_=lt_d[:, :])
            nc.sync.dma_start(out=pow_sb[:, :], in_=pow_d[:, :])
            nc.sync.dma_start(out=xt_sb[:, :], in_=xt_d[:, :])
            for i in range(8):
                nc.sync.dma_start(out=w2_sb[i][:, :],
                                  in_=w2_d[:, ds(2048 * i, 2048)])

            # ---- carries: c_t = s[tile_start t] for all 8 tiles ----
            with tc.tile_pool(name="psum_c", bufs=1, space="PSUM") as psum_c:
                c0_ps = psum_c.tile([1, P], F32)
                nc.tensor.matmul(c0_ps[:, :], lhsT=hw_sb[:, 0:1],
                                 rhs=halo_sb[:, 0, :], start=True, stop=False)
                nc.tensor.matmul(c0_ps[:, :], lhsT=hw_sb[:, 1:2],
                                 rhs=halo_sb[:, 1, :], start=False, stop=True)
                vps_a = psum_c.tile([1, 4 * P], F32, tag="vps_a")
                vps_b = psum_c.tile([1, 4 * P], F32, tag="vps_b")
                nc.tensor.matmul(vps_a[:, :], lhsT=vw_sb[:, :],
                                 rhs=x_sb[:, 0:4, :], start=True, stop=True)
                nc.tensor.matmul(vps_b[:, :], lhsT=vw_sb[:, :],
                                 rhs=x_sb[:, 4:8, :], start=True, stop=True)
                nc.scalar.copy(c0_sb[:, :], c0_ps[:, :])
                nc.scalar.copy(va_sb[:, :], vps_a[:, :])
                nc.scalar.copy(vb_sb[:, :], vps_b[:, :])
                nc.scalar.dma_start(out=v9_sb[0:1, :], in_=c0_sb[:, :])
                nc.scalar.dma_start(out=v9_sb[1:5, :], in_=va_sb[:, :])
                nc.scalar.dma_start(out=v9_sb[5:9, :], in_=vb_sb[:, :])
                c_ps = psum_c.tile([NT, P], F32, tag="c_ps")
                nc.tensor.matmul(c_ps[:, :], lhsT=lt9_sb[:, :],
                                 rhs=v9_sb[:, :], start=True, stop=True)
                nc.scalar.copy(c8_sb[:, :], c_ps[:, :])
                nc.scalar.dma_start(out=c_all[:, :], in_=c8_sb[:, :])

            # ---- s tiles: s = L @ x_t + pow (x) c_t ----
            with tc.tile_pool(name="psum_s", bufs=2, space="PSUM") as psum_s:
                for t in range(NT):
                    sp = psum_s.tile([P, P], F32)
                    nc.tensor.matmul(sp[:, :], lhsT=lt_sb[:, :],
                                     rhs=x_sb[:, t, :], start=True, stop=False)
                    nc.tensor.matmul(sp[:, :], lhsT=pow_sb[:, :],
                                     rhs=c_all[:, ts(t, P)], start=False, stop=True)
                    nc.scalar.copy(s_sb[:, t, :], sp[:, :])

            # ---- main: pass1 matmuls + multi-engine drain ----
            with tc.tile_pool(name="psum_y", bufs=4, space="PSUM") as psum_y, \
                 tc.tile_pool(name="tmp_pool", bufs=6) as tmp_pool, \
                 tc.tile_pool(name="part_pool", bufs=2) as part_pool:
                for t in range(NT):
                    xt_t = xt_sb[:, ts(t, P)]
                    partials = part_pool.tile([P, NG, P], FP16)
                    for g in range(NG):
                        e0 = g * EG
                        yp = psum_y.tile([P, EG, P], F32)   # 2 banks
                        for h in range(2):
                            nc.tensor.matmul(
                                yp[:, 4 * h:4 * h + 4, :], lhsT=xt_t,
                                rhs=w2_sb[g // 2][:, ds((g % 2) * 1024 + 512 * h, 512)],
                                start=True, stop=True)
                        tmp = tmp_pool.tile([P, 15, P], FP16)
                        if MULT[g] == "T":
                            sbc = s_sb[:, t, e0:e0 + EG, None].broadcast_to(
                                (P, EG, P))
                            nc.vector.tensor_tensor(
                                out=tmp[:, 0:EG, :], in0=yp[:, :, :], in1=sbc,
                                op=mult)
                        else:
                            for j in range(EG):
                                nc.scalar.mul(tmp[:, j, :], yp[:, j, :],
                                              s_sb[:, t, e0 + j:e0 + j + 1])
                        tree = nc.vector if TREE[g] == "D" else nc.gpsimd
                        tree.tensor_add(tmp[:, 8:12, :], tmp[:, 0:4, :],
                                        tmp[:, 4:8, :])
                        tree.tensor_add(tmp[:, 12:14, :], tmp[:, 8:10, :],
                                        tmp[:, 10:12, :])
                        tree.tensor_add(partials[:, g, :], tmp[:, 12, :],
                                        tmp[:, 13, :])
                    # tile-level tree (DVE, fp16 2x; last add emits fp32)
                    pt = part_pool.tile([P, 14, P], FP16, tag="ptree")
                    nc.vector.tensor_add(pt[:, 0:8, :], partials[:, 0:8, :],
                                         partials[:, 8:16, :])
                    nc.vector.tensor_add(pt[:, 8:12, :], pt[:, 0:4, :],
                                         pt[:, 4:8, :])
                    nc.vector.tensor_add(pt[:, 12:14, :], pt[:, 8:10, :],
                                         pt[:, 10:12, :])
                    nc.vector.tensor_add(out_sb[:, t, :], pt[:, 12, :],
                                         pt[:, 13, :])
                    nc.sync.dma_start(out=out_d[:, t, :], in_=out_sb[:, t, :])

    nc.finalize()
    return nc


def _get_nc():
    if "nc" not in _CACHE:
        _CACHE["nc"] = _build_nc()
    return _CACHE["nc"]


def kernel(x, concept_map, _trace=False):
    global LAST_RESULTS
    x = np.asarray(x, dtype=np.float32)
    cm = np.asarray(concept_map, dtype=np.float32)
    assert x.shape == (B, S, D) and cm.shape == (D, D, D)

    consts = _host_constants()
    # W2[d, e*128+f] = cm[f, d, e]
    w2 = np.ascontiguousarray(
        np.transpose(cm, (1, 2, 0)).reshape(D, D * D)).astype(ml_dtypes.bfloat16)

    in_maps = []
    for core in range(NCORES):
        b, half = divmod(core, 2)
        lo = half * CHUNK
        xc = x[b, lo:lo + CHUNK]                          # [1024, 128]
        x_il = np.ascontiguousarray(
            xc.reshape(NT, P, D).transpose(1, 0, 2))
        xt = np.ascontiguousarray(xc.T).astype(ml_dtypes.bfloat16)  # [d, p]
        if half == 0:
            halo = np.zeros((P, 2, D), dtype=np.float32)
        else:
            h = x[b, lo - HALO:lo]                        # [256, 128]
            halo = np.ascontiguousarray(h.reshape(2, P, D).transpose(1, 0, 2))
        in_maps.append({
            "x": x_il, "xt": xt, "halo": halo, "w2": w2, **consts,
        })

    nc = _get_nc()
    res = run_bass_kernel_spmd(nc, in_maps, list(range(NCORES)), trace=_trace)
    LAST_RESULTS = res

    out = np.empty((B, S, D), dtype=np.float32)
    for core in range(NCORES):
        b, half = divmod(core, 2)
        o = res.results[core]["out"]                      # [p, t, f]
        out[b, half * CHUNK:(half + 1) * CHUNK] = (
            o.transpose(1, 0, 2).reshape(CHUNK, D))
    return out
